# revision 4
# baseline (speedup 1.0000x reference)
"""Cartesian-product expansion kernel for Trainium2 (8 NeuronCores).

reference:
    a = repeat(emb_in, n, axis=0)       # [n*n, f]   a[k] = emb_in[k // n]
    b = tile(emb_in, (n, 1))            # [n*n, f]   b[k] = emb_in[k % n]
    w = tile(sum_weights[:, None], (n, 1))
    out = concat([a, b, w], axis=1)     # [n*n, 2f+1]

Pure data movement; the 1.09 GB f32 output is HBM-write-bound.

Sharding: row-block over i = k // n. Core c owns i in [c*256, (c+1)*256).

Per-core layout trick: for a fixed i, the output block [n, 65] has columns
32:65 (b|w) identical for every i. We keep K persistent SBUF buffers whose
b|w columns are written once; per iteration only the 32 "a" columns are
re-broadcast (DVE), then the whole buffer is DMA'd to DRAM contiguously.
j-rows map to partitions as j = p*16 + t so each partition emits one
contiguous DRAM segment per i-block (4160 B) — large-descriptor, full-rate
HBM writes.
"""

import numpy as np

N = 2048          # rows of emb_in
F = 32            # features
ROW = 2 * F + 1   # 65 output columns
P = 128           # SBUF partitions
NCORES = 8
IPC = N // NCORES  # 256 i-values per core
T = N // P         # 16 j-rows per partition
G = 8              # i-blocks per DMA
K = 4              # pipeline depth (persistent template buffers)
ITERS = IPC // G   # 32

_NC = None


def _build():
    global _NC
    if _NC is not None:
        return _NC
    import concourse.bass as bass  # noqa: F401
    import concourse.bacc as bacc
    import concourse.tile as tile
    from concourse import mybir

    f32 = mybir.dt.float32
    nc = bacc.Bacc("TRN2", target_bir_lowering=False, debug=False,
                   num_devices=NCORES)

    emb_slice = nc.dram_tensor("emb_slice", [IPC, F], f32, kind="ExternalInput")
    emb_full = nc.dram_tensor("emb_full", [N, F], f32, kind="ExternalInput")
    sw = nc.dram_tensor("sw", [N], f32, kind="ExternalInput")
    out = nc.dram_tensor("out", [IPC * N, ROW], f32, kind="ExternalOutput")

    # out rows: r = ((it*G + g)*P + p)*T + t ; view [it][p][g][(t c)]
    O = out[:].rearrange("(it g p t) c -> it p g (t c)", g=G, p=P, t=T)

    with tile.TileContext(nc) as tc:
        with tc.tile_pool(name="singles", bufs=1) as singles:
            # emb_full rows for this partition's j-rows: e[p, t, :] = emb_full[p*T + t]
            e = singles.tile([P, T, F], f32, tag="e")
            nc.sync.dma_start(out=e[:], in_=emb_full[:].rearrange("(p t) f -> p t f", p=P))
            swt = singles.tile([P, T], f32, tag="swt")
            nc.sync.dma_start(out=swt[:], in_=sw[:].rearrange("(p t) -> p t", p=P))

            # emb_slice replicated to every partition: es0 holds the first
            # iteration's G rows (small, lands fast so iter 0 isn't gated on
            # the full replica), es1 the rest.
            es0 = singles.tile([P, G, F], f32, tag="es0")
            nc.gpsimd.dma_start(
                out=es0[:],
                in_=emb_slice[0:G, :].unsqueeze(0).broadcast_to((P, G, F)),
            )
            es1 = singles.tile([P, IPC - G, F], f32, tag="es1")
            nc.gpsimd.dma_start(
                out=es1[:],
                in_=emb_slice[G:IPC, :].unsqueeze(0).broadcast_to((P, IPC - G, F)),
            )

            bufs = [singles.tile([P, G, T * ROW], f32, tag=f"buf{k}",
                                 name=f"buf{k}")
                    for k in range(K)]

            def fill_bw(tk):
                # static b|w columns, written once per persistent buffer
                tkv = tk[:].rearrange("p g (t c) -> p g t c", t=T)
                nc.vector.tensor_copy(
                    tkv[:, :, :, F:2 * F],
                    e[:].unsqueeze(1).broadcast_to((P, G, T, F)),
                )
                nc.vector.tensor_copy(
                    tkv[:, :, :, 2 * F:ROW],
                    swt[:].unsqueeze(1).unsqueeze(3).broadcast_to((P, G, T, 1)),
                )

            for it in range(ITERS):
                tk = bufs[it % K]
                if it < K:
                    fill_bw(tk)  # lazy: buffer k's template fills just ahead of use
                tkv = tk[:].rearrange("p g (t c) -> p g t c", t=T)
                if it == 0:
                    src = es0[:].unsqueeze(2).broadcast_to((P, G, T, F))
                else:
                    src = (es1[:, (it - 1) * G:it * G, :]
                           .unsqueeze(2).broadcast_to((P, G, T, F)))
                nc.vector.tensor_copy(tkv[:, :, :, 0:F], src)
                nc.sync.dma_start(out=O[it], in_=tk[:])

    nc.compile()
    _NC = nc
    return nc


def kernel(emb_in, sum_weights, _profile=False):
    from concourse.bass_utils import run_bass_kernel_spmd

    nc = _build()
    emb_in = np.ascontiguousarray(np.asarray(emb_in, dtype=np.float32))
    sum_weights = np.ascontiguousarray(np.asarray(sum_weights, dtype=np.float32))
    assert emb_in.shape == (N, F) and sum_weights.shape == (N,)

    in_maps = [
        {
            "emb_slice": emb_in[c * IPC:(c + 1) * IPC],
            "emb_full": emb_in,
            "sw": sum_weights,
        }
        for c in range(NCORES)
    ]
    res = run_bass_kernel_spmd(nc, in_maps, list(range(NCORES)), trace=_profile)
    full = np.concatenate([res.results[c]["out"] for c in range(NCORES)], axis=0)
    if _profile:
        return full, res.exec_time_ns
    return full


# revision 5
# speedup vs baseline: 1.0358x; 1.0358x over previous
"""Raw-Bass v4.

- replica rows 0..64 via small in-band DMA broadcast (fast ramp)
- rows 64..256 staged on partition 0 of `es` itself, then broadcast to all
  partitions via gpsimd partition_broadcast (off the SDMA engines), delayed
  into the steady-state window so it doesn't contend with ramp DVE fills
- iteration 0 split in halves; K=4 pipeline buffers
"""
import numpy as np

N = 2048
F = 32
ROW = 2 * F + 1
P = 128
NCORES = 8
IPC = N // NCORES   # 256
T = N // P          # 16
G = 8
K = 4
ITERS = IPC // G    # 32

ES0_ROWS = 64
PB_CHUNKS = [(64, 160), (160, 256)]


def _need_pb(it):
    hi = (it + 1) * G
    if hi <= ES0_ROWS:
        return 0
    for idx, (lo, chi) in enumerate(PB_CHUNKS):
        if hi <= chi:
            return idx + 1
    return len(PB_CHUNKS)


_NC = None


def _build():
    global _NC
    if _NC is not None:
        return _NC
    import concourse.bass as bass  # noqa: F401
    import concourse.bacc as bacc
    from concourse import mybir

    f32 = mybir.dt.float32
    nc = bacc.Bacc("TRN2", target_bir_lowering=False, debug=False,
                   num_devices=NCORES)

    emb_slice = nc.dram_tensor("emb_slice", [IPC, F], f32, kind="ExternalInput")
    emb_full = nc.dram_tensor("emb_full", [N, F], f32, kind="ExternalInput")
    sw = nc.dram_tensor("sw", [N], f32, kind="ExternalInput")
    out = nc.dram_tensor("out", [IPC * N, ROW], f32, kind="ExternalOutput")

    O = out[:].rearrange("(it g p t) c -> it p g (t c)", g=G, p=P, t=T)

    e = nc.alloc_sbuf_tensor("e", [P, T, F], f32)
    swt = nc.alloc_sbuf_tensor("swt", [P, T], f32)
    es = nc.alloc_sbuf_tensor("es", [P, IPC, F], f32)
    bufs = [nc.alloc_sbuf_tensor(f"buf{k}", [P, G, T * ROW], f32)
            for k in range(K)]

    vec_count = 0
    uses16 = [0] * K
    dma_plan = []   # (view_out, view_in, k, vec_ready_needed, war_count)

    with (
        nc.Block() as block,
        nc.semaphore("s_e") as s_e,
        nc.semaphore("s_sw") as s_sw,
        nc.semaphore("s_p0") as s_p0,
        nc.semaphore("s_es0") as s_es0,
        nc.semaphore("pb") as pb,
        nc.semaphore("vec_ready") as vec_ready,
        nc.semaphore("out0") as out0,
        nc.semaphore("out1") as out1,
        nc.semaphore("out2") as out2,
        nc.semaphore("out3") as out3,
    ):
        out_sems = [out0, out1, out2, out3]

        @block.scalar
        def _(scalar):
            scalar.dma_start(
                out=swt[:], in_=sw[:].rearrange("(p t) -> p t", p=P)
            ).then_inc(s_sw, 16)
            # stage replica rows 64..256 on partition 0 of es
            scalar.dma_start(
                out=es[0:1, ES0_ROWS:IPC, :],
                in_=emb_slice[ES0_ROWS:IPC, :].flatten().unsqueeze(0),
            ).then_inc(s_p0, 16)

        @block.gpsimd
        def _(gpsimd):
            gpsimd.dma_start(
                out=es[:, 0:ES0_ROWS, :],
                in_=emb_slice[0:ES0_ROWS, :].unsqueeze(0)
                .broadcast_to((P, ES0_ROWS, F)),
            ).then_inc(s_es0, 16)
            gpsimd.wait_ge(s_p0, 16)
            # delay the broadcasts into steady state (DVE is well ahead by then)
            gpsimd.wait_ge(vec_ready, 7)
            for lo, hi in PB_CHUNKS:
                gpsimd.partition_broadcast(
                    es[:, lo:hi, :], es[0:1, lo:hi, :]
                ).then_inc(pb, 1)

        @block.vector
        def _(vector):
            nonlocal vec_count
            vector.wait_ge(s_e, 16)
            state = {"pb": 0, "sw": False, "es0": False}

            def fills(k, g_lo, g_hi, i_lo):
                nonlocal vec_count
                tkv = bufs[k][:].rearrange("p g (t c) -> p g t c", t=T)
                gs = g_hi - g_lo
                vector.tensor_copy(
                    tkv[:, g_lo:g_hi, :, F:2 * F],
                    e[:].unsqueeze(1).broadcast_to((P, gs, T, F)),
                )
                if not state["sw"]:
                    vector.wait_ge(s_sw, 16)
                    state["sw"] = True
                vector.tensor_copy(
                    tkv[:, g_lo:g_hi, :, 2 * F:ROW],
                    swt[:].unsqueeze(1).unsqueeze(3).broadcast_to((P, gs, T, 1)),
                )
                if not state["es0"]:
                    vector.wait_ge(s_es0, 16)
                    state["es0"] = True
                vector.tensor_copy(
                    tkv[:, g_lo:g_hi, :, 0:F],
                    es[:, i_lo:i_lo + gs, :].unsqueeze(2).broadcast_to((P, gs, T, F)),
                ).then_inc(vec_ready, 1)
                vec_count += 1

            # iteration 0 in halves (buffer 0)
            H = G // 2
            for h in range(2):
                fills(0, h * H, (h + 1) * H, h * H)
                dma_plan.append((O[0][:, h * H:(h + 1) * H, :],
                                 bufs[0][:, h * H:(h + 1) * H, :],
                                 0, vec_count, None))
                uses16[0] += 16
            for it in range(1, ITERS):
                k = it % K
                war = uses16[k] if it >= K else None
                if war is not None:
                    vector.wait_ge(out_sems[k], war)
                need = _need_pb(it)
                if need > state["pb"]:
                    vector.wait_ge(pb, need)
                    state["pb"] = need
                if it < K:
                    fills(k, 0, G, it * G)
                else:
                    tkv = bufs[k][:].rearrange("p g (t c) -> p g t c", t=T)
                    vector.tensor_copy(
                        tkv[:, :, :, 0:F],
                        es[:, it * G:(it + 1) * G, :]
                        .unsqueeze(2).broadcast_to((P, G, T, F)),
                    ).then_inc(vec_ready, 1)
                    vec_count += 1
                dma_plan.append((O[it], bufs[k][:], k, vec_count, uses16[k]))
                uses16[k] += 16

        @block.sync
        def _(sync):
            sync.dma_start(
                out=e[:], in_=emb_full[:].rearrange("(p t) f -> p t f", p=P)
            ).then_inc(s_e, 16)
            last_war = [0] * K
            for view_out, view_in, k, need_vec, war in dma_plan:
                sync.wait_ge(vec_ready, need_vec)
                if war is not None and war > last_war[k]:
                    sync.wait_ge(out_sems[k], war)
                    last_war[k] = war
                sync.dma_start(out=view_out, in_=view_in).then_inc(out_sems[k], 16)
            for k in range(K):
                sync.wait_ge(out_sems[k], uses16[k])

    nc.compile()
    _NC = nc
    return nc


def kernel(emb_in, sum_weights, _profile=False):
    from concourse.bass_utils import run_bass_kernel_spmd

    nc = _build()
    emb_in = np.ascontiguousarray(np.asarray(emb_in, dtype=np.float32))
    sum_weights = np.ascontiguousarray(np.asarray(sum_weights, dtype=np.float32))
    in_maps = [
        {
            "emb_slice": emb_in[c * IPC:(c + 1) * IPC],
            "emb_full": emb_in,
            "sw": sum_weights,
        }
        for c in range(NCORES)
    ]
    res = run_bass_kernel_spmd(nc, in_maps, list(range(NCORES)), trace=_profile)
    full = np.concatenate([res.results[c]["out"] for c in range(NCORES)], axis=0)
    if _profile:
        return full, res.exec_time_ns
    return full
